# revision 33
# baseline (speedup 1.0000x reference)
"""APPNP tree-GNN propagation on 8 TRN2 NeuronCores.

Model (dims hardcoded; see build()):
    d      = rowsum(adj) ; ds = d^-1/2
    a_norm = ds[:,None] * adj * ds[None,:]
    h0     = relu(nodes_encs @ W + b)
    h      = (1-alpha) * (a_norm @ h) + alpha*h0     (x NLAYER)

Distribution: row-shard adj and nodes_encs across 8 cores (1024 rows each).
The host passes each core its row-shard TRANSPOSED (so the contraction dim
lands on SBUF partitions; no on-device transposes needed).  Key trick: adj
entries are exactly {0,1}, so the f32->bf16 cast of the raw adjacency is
lossless; A stays resident in SBUF in bf16 (16 MB/core) and is read from
HBM exactly once.  The ds scalings fold into per-strip vector epilogues:
with g = ds*h,  g' = (1-a)*ds^2*(A@g) + a*ds*h0 = s1*psum + K0d.
Per layer the 256-wide g is AllGather'd across the 8 cores.
"""

import os
import sys
import types
import contextlib

import numpy as np

for _p in (
    "/root/.axon_site",
    "/root/.axon_site/_ro/trn_rl_repo",
    "/root/.axon_site/_ro/pypackages",
    "/opt/trn_rl_repo",
    "/opt/pypackages",
):
    if os.path.isdir(_p) and _p not in sys.path:
        sys.path.append(_p)

import concourse.bass as bass  # noqa: E402
import concourse.mybir as mybir  # noqa: E402
import concourse.tile as tile  # noqa: E402
from concourse import bacc  # noqa: E402
from concourse.bass_utils import run_bass_kernel_spmd  # noqa: E402

F32 = mybir.dt.float32
BF16 = mybir.dt.bfloat16
FP8 = mybir.dt.float8e4
ALU = mybir.AluOpType
ACTF = mybir.ActivationFunctionType

N, EMB, HID = 8192, 1024, 256
NLAYER, ALPHA, NCORES = 6, 0.2, 8

LAST_EXEC_NS = None
LAST_TRACE = None


def _install_ntff_hook():
    """antenv.axon_hooks is absent in this image; rebuild it from the boot
    helpers so run_bass_kernel_spmd(trace=True) can capture NTFF profiles."""
    try:
        from antenv.axon_hooks import get_axon_ntff_profile_hook  # noqa: F401

        return
    except ImportError:
        pass
    try:
        import antenv
        from trn_agent_boot.trn_boot import _ntff_profile_via_ctypes

        hook = _ntff_profile_via_ctypes("/opt/axon/libaxon_pjrt.so")
        mod = types.ModuleType("antenv.axon_hooks")
        _h = [hook]
        mod.get_axon_ntff_profile_hook = lambda: _h[0]
        mod.set_axon_ntff_profile_hook = lambda h: _h.__setitem__(0, h)
        sys.modules["antenv.axon_hooks"] = mod
        antenv.axon_hooks = mod
    except Exception:
        pass


def _build_body(tc, nc, aps, n, emb, hid, rows, nlayer, alpha, ncores, a_dtype=BF16):
    at_d, xt_d, w_d, out_d = aps
    KT = n // 128  # contraction tiles over all nodes
    EK = emb // 128  # contraction tiles over embedding dim
    MS = rows // 128  # output row strips per core
    DCH = max(1, rows // 512)  # 512-wide chunks for the degree matmul
    DW = min(512, rows)
    CH = min(8, KT)  # chunks for the gathered-h reload
    TPC = KT // CH
    STW = max(rows, hid)  # stage tile width

    with (
        tc.tile_pool(name="big", bufs=1) as big,
        tc.tile_pool(name="stage", bufs=3) as stage,
        tc.tile_pool(name="eout", bufs=3) as eout,
        tc.tile_pool(name="pstrip", bufs=8, space="PSUM") as pstrip,
        tc.tile_pool(name="dram", bufs=1, space="DRAM") as dram,
    ):
        # ---- resident SBUF tensors --------------------------------------
        A_sb = big.tile([128, KT, rows], a_dtype)  # raw 0/1 adjacency (exact)
        h_sb = big.tile([128, 2, KT, hid], BF16)  # gathered g (ping/pong)
        W_sb = big.tile([128, EK, hid], BF16)
        h0b = big.tile([128, MS, hid], BF16)  # relu(X@W+b) own rows
        K0d = big.tile([128, MS, hid], F32)  # alpha * ds * h0 (own rows)
        d_t = big.tile([128, MS], F32)  # degree, strip-major
        s3 = big.tile([128, MS], F32)  # sqrt(d)  (= 1/ds)
        ds_t = big.tile([128, MS], F32)  # d^-1/2
        dsq = big.tile([128, MS], F32)
        s1 = big.tile([128, MS], F32)  # (1-alpha) * ds^2
        ones_w = big.tile([128, 1], a_dtype)
        d_row = big.tile([1, rows], F32)
        ds_full_t = big.tile([128, KT], F32)

        # ---- DRAM bounce buffers ---------------------------------------
        HS = (MS + 1) // 2  # strips in the first allgather half
        d_dram = dram.tile([1, rows], F32)
        ds_dram = dram.tile([1, rows], F32)
        ds_full_dram = dram.tile(
            [ncores, rows], F32, addr_space="Shared", name="ds_full_dram"
        )
        g_local_a = dram.tile([HS * 128, hid], BF16)
        g_local_b = None
        if MS > HS:
            g_local_b = dram.tile(
                [(MS - HS) * 128, hid], BF16, name="g_local_b"
            )

        # ---- PSUM: one pool of 8 single-bank slots ---------------------
        d_ps = [
            pstrip.tile([1, DW], F32, tag="strip", name=f"d_ps{c}")
            for c in range(DCH)
        ]

        nc.vector.memset(ones_w, 1.0)

        # ---- W / b load ------------------------------------------------
        for kt in range(EK):
            wst = stage.tile([128, hid], F32, tag="sm", name=f"wst{kt}", bufs=2)
            nc.scalar.dma_start(wst, w_d[kt * 128 : (kt + 1) * 128, :])
            nc.vector.tensor_copy(W_sb[:, kt, :], wst)

        def ag_half(tag, half, dst):
            """AllGather one strip-half and reload it into h_sb[:, dst]."""
            g_loc = g_local_a if half == 0 else g_local_b
            nstr = HS if half == 0 else MS - HS
            base = 0 if half == 0 else HS
            gathered = dram.tile(
                [ncores * nstr * 128, hid], BF16, addr_space="Shared",
                name=f"gath_{tag}_{half}", tag=f"gath_{tag}_{half}",
            )
            nc.gpsimd.collective_compute(
                "AllGather",
                ALU.bypass,
                replica_groups=[list(range(ncores))],
                ins=[g_loc.opt()],
                outs=[gathered.opt()],
            )
            rpr = nstr * 128  # rows per rank in this gather
            for r in range(ncores):
                nc.gpsimd.dma_start(
                    h_sb[:, dst, r * MS + base : r * MS + base + nstr, :],
                    gathered[r * rpr : (r + 1) * rpr, :].rearrange(
                        "(k p) h -> p k h", p=128
                    ),
                )

        def k_order():
            """a-half k-tiles first (available right after ag half 0)."""
            ka = [r * MS + t for r in range(ncores) for t in range(HS)]
            kb = [r * MS + t for r in range(ncores) for t in range(HS, MS)]
            return ka + kb

        def store_g(m, g):
            if m < HS:
                nc.gpsimd.dma_start(g_local_a[m * 128 : (m + 1) * 128, :], g)
            else:
                mm = m - HS
                nc.gpsimd.dma_start(g_local_b[mm * 128 : (mm + 1) * 128, :], g)
            if m == HS - 1:
                ag_half(store_g.tag, 0, store_g.dst)
            if m == MS - 1 and g_local_b is not None:
                ag_half(store_g.tag, 1, store_g.dst)

        # ---- propagation layers ----------------------------------------
        # ---- h0 = relu(X @ W + b), strip-group passes so only <=4 psum
        # banks are in flight (degree chains use 2 more) -----------------
        GRP = MS if MS <= 8 else 4
        for g0 in range(0, MS, GRP):
            strips = range(g0, min(g0 + GRP, MS))
            psums = {m: pstrip.tile([128, hid], F32, tag="strip", name=f"ps_h0_{m}")
                     for m in strips}
            for kt in range(EK):
                xst = stage.tile([128, STW], F32, tag="xst", name=f"xst{g0}_{kt}", bufs=2)
                nc.scalar.dma_start(
                    xst[:, :rows], xt_d[kt * 128 : (kt + 1) * 128, :]
                )
                xbf = stage.tile([128, STW], BF16, tag="stgbf", name=f"xbf{g0}_{kt}", bufs=2)
                nc.vector.tensor_copy(xbf[:, :rows], xst[:, :rows])
                for m in strips:
                    nc.tensor.matmul(
                        psums[m],
                        lhsT=xbf[:, m * 128 : (m + 1) * 128],
                        rhs=W_sb[:, kt, :],
                        start=(kt == 0),
                        stop=(kt == EK - 1),
                    )
            for m in strips:
                nc.scalar.activation(h0b[:, m, :], psums[m], ACTF.Relu)
                store_g.tag = "l0"
                store_g.dst = 0
                store_g(m, h0b[:, m, :])

        # ---- adjacency load + lossless bf16 cast + degree matmul -------
        LG = 4 if KT % 4 == 0 else 1  # k-tiles per load DMA
        for kg in range(KT // LG):
            ast = stage.tile(
                [128, LG, rows], F32, tag="ag", name=f"ast{kg}", bufs=2
            )
            nc.sync.dma_start(
                ast,
                at_d[kg * LG * 128 : (kg + 1) * LG * 128, :].rearrange(
                    "(t p) r -> p t r", p=128
                ),
            )
            hl = max(1, LG // 2)
            nc.vector.tensor_copy(A_sb[:, kg * LG : kg * LG + hl, :], ast[:, :hl, :])
            if LG > 1:
                nc.vector.tensor_copy(
                    A_sb[:, kg * LG + hl : (kg + 1) * LG, :], ast[:, hl:, :]
                )
            for t in range(LG):
                k = kg * LG + t
                for c in range(DCH):
                    nc.tensor.matmul(
                        d_ps[c][0:1, :],
                        lhsT=ones_w,
                        rhs=A_sb[:, k, c * DW : (c + 1) * DW],
                        start=(k == 0),
                        stop=(k == KT - 1),
                    )

        # ---- degree -> scaling vectors ---------------------------------
        # global ds first (critical path): sqrt straight from PSUM
        dsr = big.tile([1, rows], F32)
        for c in range(DCH):
            nc.scalar.sqrt(dsr[:, c * DW : (c + 1) * DW], d_ps[c][0:1, :])
        nc.vector.reciprocal(dsr, dsr)
        nc.sync.dma_start(ds_dram, dsr)
        # local strip-major ds derived from the same row
        nc.sync.dma_start(ds_t, ds_dram[0, :].rearrange("(m p) -> p m", p=128))
        nc.vector.reciprocal(s3, ds_t)  # sqrt(d) = 1/ds
        nc.vector.tensor_mul(dsq, ds_t, ds_t)
        nc.vector.tensor_scalar_mul(s1, dsq, 1.0 - alpha)
        nc.gpsimd.collective_compute(
            "AllGather",
            ALU.bypass,
            replica_groups=[list(range(ncores))],
            ins=[ds_dram.opt()],
            outs=[ds_full_dram.opt()],
        )
        nc.sync.dma_start(
            ds_full_t,
            ds_full_dram[:, :].rearrange("r (m p) -> p (r m)", p=128),
        )

        # ---- K0d = alpha*ds*h0 ; scale gathered h0 into g0 buffer ------
        for m in range(MS):
            nc.vector.tensor_scalar(
                K0d[:, m, :], h0b[:, m, :], ds_t[:, m : m + 1], alpha,
                ALU.mult, ALU.mult,
            )
        for i, k in enumerate(k_order()):
            if i % 2 == 0:
                nc.vector.tensor_scalar_mul(
                    h_sb[:, 1, k, :], h_sb[:, 0, k, :], ds_full_t[:, k : k + 1]
                )
            else:
                nc.scalar.mul(
                    h_sb[:, 1, k, :], h_sb[:, 0, k, :], ds_full_t[:, k : k + 1]
                )

        ka = [r * MS + t for r in range(ncores) for t in range(HS)]
        kb = [r * MS + t for r in range(ncores) for t in range(HS, MS)]
        for layer in range(1, nlayer + 1):
            last = layer == nlayer
            store_g.tag = f"l{layer}"
            store_g.dst = (layer + 1) % 2
            src_buf = layer % 2
            psums = {}
            for m in range(MS):
                psums[m] = pstrip.tile(
                    [128, hid], F32, tag="strip", name=f"ps_{layer}_{m}"
                )
                for i, k in enumerate(ka):
                    nc.tensor.matmul(
                        psums[m],
                        lhsT=A_sb[:, k, m * 128 : (m + 1) * 128],
                        rhs=h_sb[:, src_buf, k, :],
                        start=(i == 0),
                        stop=(not kb and i == len(ka) - 1),
                    )
            for m in range(MS):
                psum_m = psums[m]
                for i, k in enumerate(kb):
                    nc.tensor.matmul(
                        psum_m,
                        lhsT=A_sb[:, k, m * 128 : (m + 1) * 128],
                        rhs=h_sb[:, src_buf, k, :],
                        start=False,
                        stop=(i == len(kb) - 1),
                    )
                if not last:
                    g = eout.tile([128, hid], BF16, tag="g", name=f"g{layer}_{m}")
                    nc.vector.scalar_tensor_tensor(
                        g, psum_m, s1[:, m : m + 1], K0d[:, m, :],
                        ALU.mult, ALU.add,
                    )
                    store_g(m, g)
                else:
                    hf = stage.tile([128, hid], F32, tag="sm", name=f"hf{m}", bufs=2)
                    nc.vector.scalar_tensor_tensor(
                        hf, psum_m, s1[:, m : m + 1], K0d[:, m, :],
                        ALU.mult, ALU.add,
                    )
                    ho = eout.tile([128, hid], F32, tag="ho", name=f"ho{m}", bufs=2)
                    nc.vector.tensor_scalar_mul(ho, hf, s3[:, m : m + 1])
                    nc.sync.dma_start(out_d[m * 128 : (m + 1) * 128, :], ho)


def build(n=N, emb=EMB, hid=HID, nlayer=NLAYER, alpha=ALPHA, ncores=NCORES, a_dtype=BF16):
    rows = n // ncores
    embp = emb + 128  # bias folded in as an extra contraction block
    nc = bacc.Bacc("TRN2", target_bir_lowering=False, debug=False, num_devices=ncores)
    at_d = nc.dram_tensor("at", [n, rows], F32, kind="ExternalInput").ap()
    xt_d = nc.dram_tensor("xt", [embp, rows], F32, kind="ExternalInput").ap()
    w_d = nc.dram_tensor("w", [embp, hid], F32, kind="ExternalInput").ap()
    out_d = nc.dram_tensor("out", [rows, hid], F32, kind="ExternalOutput").ap()
    with tile.TileContext(nc) as tc:
        _build_body(
            tc, nc, (at_d, xt_d, w_d, out_d), n, embp, hid, rows, nlayer,
            alpha, ncores, a_dtype=a_dtype,
        )
    nc.compile()
    return nc


def make_in_maps(nodes_encs, W, b, adj, ncores=NCORES):
    n = adj.shape[0]
    rows = n // ncores
    emb = np.asarray(W).shape[0]
    hid = np.asarray(W).shape[1]
    # bias folded into an extra 128-row contraction block:
    # X' = [X | e0-block], W' = [W ; b-row-block]
    Wp = np.zeros((emb + 128, hid), dtype=np.float32)
    Wp[:emb] = np.asarray(W, dtype=np.float32)
    Wp[emb] = np.asarray(b, dtype=np.float32).reshape(-1)
    in_maps = []
    for c in range(ncores):
        sl = slice(c * rows, (c + 1) * rows)
        xtp = np.zeros((emb + 128, rows), dtype=np.float32)
        xtp[:emb] = np.asarray(nodes_encs)[sl, :].T
        xtp[emb] = 1.0
        in_maps.append(
            {
                "at": np.ascontiguousarray(np.asarray(adj)[sl, :].T, dtype=np.float32),
                "xt": xtp,
                "w": Wp,
            }
        )
    return in_maps


def kernel(nodes_encs, W, b, adj, trace=True):
    global LAST_EXEC_NS, LAST_TRACE
    _install_ntff_hook()
    a_dtype = BF16 if os.environ.get("APPNP_A_BF16") else FP8
    nc = build(a_dtype=a_dtype)
    in_maps = make_in_maps(nodes_encs, W, b, adj)
    res = None
    if trace:
        try:
            # warmup execution absorbs NEFF-load / core-start skew
            run_bass_kernel_spmd(
                nc, in_maps, core_ids=list(range(NCORES)), trace=False
            )
            res = run_bass_kernel_spmd(
                nc, in_maps, core_ids=list(range(NCORES)), trace=True
            )
        except Exception:
            res = None
    if res is None:
        res = run_bass_kernel_spmd(
            nc, in_maps, core_ids=list(range(NCORES)), trace=False
        )
    LAST_EXEC_NS = res.exec_time_ns
    LAST_TRACE = getattr(res, "instructions_and_trace", None)
    out = np.concatenate([res.results[c]["out"] for c in range(NCORES)], axis=0)
    return out.astype(np.float32)


# revision 35
# speedup vs baseline: 1.0323x; 1.0323x over previous
"""APPNP tree-GNN propagation on 8 TRN2 NeuronCores.

Model (dims hardcoded; see build()):
    d      = rowsum(adj) ; ds = d^-1/2
    a_norm = ds[:,None] * adj * ds[None,:]
    h0     = relu(nodes_encs @ W + b)
    h      = (1-alpha) * (a_norm @ h) + alpha*h0     (x NLAYER)

Distribution: row-shard adj and nodes_encs across 8 cores (1024 rows each).
The host passes each core its row-shard TRANSPOSED (so the contraction dim
lands on SBUF partitions; no on-device transposes needed).  Key trick: adj
entries are exactly {0,1}, so the f32->bf16 cast of the raw adjacency is
lossless; A stays resident in SBUF in bf16 (16 MB/core) and is read from
HBM exactly once.  The ds scalings fold into per-strip vector epilogues:
with g = ds*h,  g' = (1-a)*ds^2*(A@g) + a*ds*h0 = s1*psum + K0d.
Per layer the 256-wide g is AllGather'd across the 8 cores.
"""

import os
import sys
import types
import contextlib

import numpy as np

for _p in (
    "/root/.axon_site",
    "/root/.axon_site/_ro/trn_rl_repo",
    "/root/.axon_site/_ro/pypackages",
    "/opt/trn_rl_repo",
    "/opt/pypackages",
):
    if os.path.isdir(_p) and _p not in sys.path:
        sys.path.append(_p)

import concourse.bass as bass  # noqa: E402
import concourse.mybir as mybir  # noqa: E402
import concourse.tile as tile  # noqa: E402
from concourse import bacc  # noqa: E402
from concourse.bass_utils import run_bass_kernel_spmd  # noqa: E402

F32 = mybir.dt.float32
BF16 = mybir.dt.bfloat16
FP8 = mybir.dt.float8e4
ALU = mybir.AluOpType
ACTF = mybir.ActivationFunctionType

N, EMB, HID = 8192, 1024, 256
NLAYER, ALPHA, NCORES = 6, 0.2, 8

LAST_EXEC_NS = None
LAST_TRACE = None


def _install_ntff_hook():
    """antenv.axon_hooks is absent in this image; rebuild it from the boot
    helpers so run_bass_kernel_spmd(trace=True) can capture NTFF profiles."""
    try:
        from antenv.axon_hooks import get_axon_ntff_profile_hook  # noqa: F401

        return
    except ImportError:
        pass
    try:
        import antenv
        from trn_agent_boot.trn_boot import _ntff_profile_via_ctypes

        hook = _ntff_profile_via_ctypes("/opt/axon/libaxon_pjrt.so")
        mod = types.ModuleType("antenv.axon_hooks")
        _h = [hook]
        mod.get_axon_ntff_profile_hook = lambda: _h[0]
        mod.set_axon_ntff_profile_hook = lambda h: _h.__setitem__(0, h)
        sys.modules["antenv.axon_hooks"] = mod
        antenv.axon_hooks = mod
    except Exception:
        pass


def _build_body(tc, nc, aps, n, emb, hid, rows, nlayer, alpha, ncores, a_dtype=BF16):
    at_d, xt_d, w_d, out_d = aps
    KT = n // 128  # contraction tiles over all nodes
    EK = emb // 128  # contraction tiles over embedding dim
    MS = rows // 128  # output row strips per core
    DCH = max(1, rows // 512)  # 512-wide chunks for the degree matmul
    DW = min(512, rows)
    CH = min(8, KT)  # chunks for the gathered-h reload
    TPC = KT // CH
    STW = max(rows, hid)  # stage tile width

    with (
        tc.tile_pool(name="big", bufs=1) as big,
        tc.tile_pool(name="stage", bufs=3) as stage,
        tc.tile_pool(name="eout", bufs=3) as eout,
        tc.tile_pool(name="pstrip", bufs=8, space="PSUM") as pstrip,
        tc.tile_pool(name="dram", bufs=1, space="DRAM") as dram,
    ):
        # ---- resident SBUF tensors --------------------------------------
        A_sb = big.tile([128, KT, rows], a_dtype)  # raw 0/1 adjacency (exact)
        h_sb = big.tile([128, 2, KT, hid], BF16)  # gathered g (ping/pong)
        W_sb = big.tile([128, EK, hid], BF16)
        h0b = big.tile([128, MS, hid], BF16)  # relu(X@W+b) own rows
        K0d = big.tile([128, MS, hid], F32)  # alpha * ds * h0 (own rows)
        d_t = big.tile([128, MS], F32)  # degree, strip-major
        s3 = big.tile([128, MS], F32)  # sqrt(d)  (= 1/ds)
        ds_t = big.tile([128, MS], F32)  # d^-1/2
        dsq = big.tile([128, MS], F32)
        s1 = big.tile([128, MS], F32)  # (1-alpha) * ds^2
        ones_w = big.tile([128, 1], a_dtype)
        d_row = big.tile([1, rows], F32)
        ds_full_t = big.tile([128, KT], F32)

        # ---- DRAM bounce buffers ---------------------------------------
        HS = (MS + 1) // 2  # strips in the first allgather half
        d_dram = dram.tile([1, rows], F32)
        ds_dram = dram.tile([1, rows], F32)
        ds_full_dram = dram.tile(
            [ncores, rows], F32, addr_space="Shared", name="ds_full_dram"
        )
        g_local_a = dram.tile([HS * 128, hid], BF16)
        g_local_b = None
        if MS > HS:
            g_local_b = dram.tile(
                [(MS - HS) * 128, hid], BF16, name="g_local_b"
            )

        # ---- PSUM: one pool of 8 single-bank slots ---------------------
        d_ps = [
            pstrip.tile([1, DW], F32, tag="strip", name=f"d_ps{c}")
            for c in range(DCH)
        ]

        nc.vector.memset(ones_w, 1.0)

        # ---- W / b load ------------------------------------------------
        for kt in range(EK):
            wst = stage.tile([128, hid], F32, tag="sm", name=f"wst{kt}", bufs=2)
            nc.scalar.dma_start(wst, w_d[kt * 128 : (kt + 1) * 128, :])
            nc.vector.tensor_copy(W_sb[:, kt, :], wst)

        def ag_half(tag, half, dst):
            """AllGather one strip-half and reload it into h_sb[:, dst]."""
            g_loc = g_local_a if half == 0 else g_local_b
            nstr = HS if half == 0 else MS - HS
            base = 0 if half == 0 else HS
            gathered = dram.tile(
                [ncores * nstr * 128, hid], BF16, addr_space="Shared",
                name=f"gath_{tag}_{half}", tag=f"gath_{tag}_{half}",
            )
            nc.gpsimd.collective_compute(
                "AllGather",
                ALU.bypass,
                replica_groups=[list(range(ncores))],
                ins=[g_loc.opt()],
                outs=[gathered.opt()],
            )
            rpr = nstr * 128  # rows per rank in this gather
            for r in range(ncores):
                nc.gpsimd.dma_start(
                    h_sb[:, dst, r * MS + base : r * MS + base + nstr, :],
                    gathered[r * rpr : (r + 1) * rpr, :].rearrange(
                        "(k p) h -> p k h", p=128
                    ),
                )

        def k_order():
            """a-half k-tiles first (available right after ag half 0)."""
            ka = [r * MS + t for r in range(ncores) for t in range(HS)]
            kb = [r * MS + t for r in range(ncores) for t in range(HS, MS)]
            return ka + kb

        def store_g(m, g):
            if m < HS:
                nc.gpsimd.dma_start(g_local_a[m * 128 : (m + 1) * 128, :], g)
            else:
                mm = m - HS
                nc.gpsimd.dma_start(g_local_b[mm * 128 : (mm + 1) * 128, :], g)
            if m == HS - 1:
                ag_half(store_g.tag, 0, store_g.dst)
            if m == MS - 1 and g_local_b is not None:
                ag_half(store_g.tag, 1, store_g.dst)

        # ---- propagation layers ----------------------------------------
        # ---- h0 = relu(X @ W + b), strip-group passes so only <=4 psum
        # banks are in flight (degree chains use 2 more) -----------------
        GRP = MS if MS <= 8 else 4
        for g0 in range(0, MS, GRP):
            strips = range(g0, min(g0 + GRP, MS))
            psums = {m: pstrip.tile([128, hid], F32, tag="strip", name=f"ps_h0_{m}")
                     for m in strips}
            for kt in range(EK):
                xst = stage.tile([128, STW], F32, tag="xst", name=f"xst{g0}_{kt}", bufs=2)
                nc.scalar.dma_start(
                    xst[:, :rows], xt_d[kt * 128 : (kt + 1) * 128, :]
                )
                xbf = stage.tile([128, STW], BF16, tag="stgbf", name=f"xbf{g0}_{kt}", bufs=2)
                nc.vector.tensor_copy(xbf[:, :rows], xst[:, :rows])
                for m in strips:
                    nc.tensor.matmul(
                        psums[m],
                        lhsT=xbf[:, m * 128 : (m + 1) * 128],
                        rhs=W_sb[:, kt, :],
                        start=(kt == 0),
                        stop=(kt == EK - 1),
                    )
            for m in strips:
                nc.scalar.activation(h0b[:, m, :], psums[m], ACTF.Relu)
                store_g.tag = "l0"
                store_g.dst = 0
                store_g(m, h0b[:, m, :])

        # ---- adjacency load + lossless bf16 cast + degree matmul -------
        LG = 4 if KT % 4 == 0 else 1  # k-tiles per load DMA
        for kg in range(KT // LG):
            ast = stage.tile(
                [128, LG, rows], F32, tag="ag", name=f"ast{kg}", bufs=2
            )
            nc.sync.dma_start(
                ast,
                at_d[kg * LG * 128 : (kg + 1) * LG * 128, :].rearrange(
                    "(t p) r -> p t r", p=128
                ),
            )
            hl = max(1, LG // 2)
            nc.vector.tensor_copy(A_sb[:, kg * LG : kg * LG + hl, :], ast[:, :hl, :])
            if LG > 1:
                nc.vector.tensor_copy(
                    A_sb[:, kg * LG + hl : (kg + 1) * LG, :], ast[:, hl:, :]
                )
            for t in range(LG):
                k = kg * LG + t
                for c in range(DCH):
                    nc.tensor.matmul(
                        d_ps[c][0:1, :],
                        lhsT=ones_w,
                        rhs=A_sb[:, k, c * DW : (c + 1) * DW],
                        start=(k == 0),
                        stop=(k == KT - 1),
                    )

        # ---- degree -> scaling vectors ---------------------------------
        for c in range(DCH):
            nc.vector.tensor_copy(d_row[:, c * DW : (c + 1) * DW], d_ps[c][0:1, :])
        # global ds first (critical path): sqrt straight from PSUM
        dsr = big.tile([1, rows], F32)
        for c in range(DCH):
            nc.scalar.sqrt(dsr[:, c * DW : (c + 1) * DW], d_ps[c][0:1, :])
        nc.vector.reciprocal(dsr, dsr)
        nc.sync.dma_start(ds_dram, dsr)
        # local strip-major ds for the epilogues
        nc.sync.dma_start(d_dram, d_row)
        nc.sync.dma_start(d_t, d_dram[0, :].rearrange("(m p) -> p m", p=128))
        nc.scalar.sqrt(s3, d_t)  # sqrt(d)
        nc.vector.reciprocal(ds_t, s3)  # d^-1/2
        nc.vector.tensor_mul(dsq, ds_t, ds_t)
        nc.vector.tensor_scalar_mul(s1, dsq, 1.0 - alpha)
        nc.gpsimd.collective_compute(
            "AllGather",
            ALU.bypass,
            replica_groups=[list(range(ncores))],
            ins=[ds_dram.opt()],
            outs=[ds_full_dram.opt()],
        )
        nc.sync.dma_start(
            ds_full_t,
            ds_full_dram[:, :].rearrange("r (m p) -> p (r m)", p=128),
        )

        # ---- K0d = alpha*ds*h0 ; scale gathered h0 into g0 buffer ------
        for m in range(MS):
            nc.vector.tensor_scalar(
                K0d[:, m, :], h0b[:, m, :], ds_t[:, m : m + 1], alpha,
                ALU.mult, ALU.mult,
            )
        for i, k in enumerate(k_order()):
            if i % 2 == 0:
                nc.vector.tensor_scalar_mul(
                    h_sb[:, 1, k, :], h_sb[:, 0, k, :], ds_full_t[:, k : k + 1]
                )
            else:
                nc.scalar.mul(
                    h_sb[:, 1, k, :], h_sb[:, 0, k, :], ds_full_t[:, k : k + 1]
                )

        ks = k_order()
        for layer in range(1, nlayer + 1):
            last = layer == nlayer
            store_g.tag = f"l{layer}"
            store_g.dst = (layer + 1) % 2
            src_buf = layer % 2
            for m in range(MS):
                psum_m = pstrip.tile(
                    [128, hid], F32, tag="strip", name=f"ps_{layer}_{m}"
                )
                for i, k in enumerate(ks):
                    nc.tensor.matmul(
                        psum_m,
                        lhsT=A_sb[:, k, m * 128 : (m + 1) * 128],
                        rhs=h_sb[:, src_buf, k, :],
                        start=(i == 0),
                        stop=(i == KT - 1),
                    )
                if not last:
                    g = eout.tile([128, hid], BF16, tag="g", name=f"g{layer}_{m}")
                    nc.vector.scalar_tensor_tensor(
                        g, psum_m, s1[:, m : m + 1], K0d[:, m, :],
                        ALU.mult, ALU.add,
                    )
                    store_g(m, g)
                else:
                    hf = stage.tile([128, hid], F32, tag="sm", name=f"hf{m}", bufs=2)
                    nc.vector.scalar_tensor_tensor(
                        hf, psum_m, s1[:, m : m + 1], K0d[:, m, :],
                        ALU.mult, ALU.add,
                    )
                    ho = eout.tile([128, hid], F32, tag="ho", name=f"ho{m}", bufs=2)
                    nc.vector.tensor_scalar_mul(ho, hf, s3[:, m : m + 1])
                    nc.sync.dma_start(out_d[m * 128 : (m + 1) * 128, :], ho)


def build(n=N, emb=EMB, hid=HID, nlayer=NLAYER, alpha=ALPHA, ncores=NCORES, a_dtype=BF16):
    rows = n // ncores
    embp = emb + 128  # bias folded in as an extra contraction block
    nc = bacc.Bacc("TRN2", target_bir_lowering=False, debug=False, num_devices=ncores)
    at_d = nc.dram_tensor("at", [n, rows], F32, kind="ExternalInput").ap()
    xt_d = nc.dram_tensor("xt", [embp, rows], F32, kind="ExternalInput").ap()
    w_d = nc.dram_tensor("w", [embp, hid], F32, kind="ExternalInput").ap()
    out_d = nc.dram_tensor("out", [rows, hid], F32, kind="ExternalOutput").ap()
    with tile.TileContext(nc) as tc:
        _build_body(
            tc, nc, (at_d, xt_d, w_d, out_d), n, embp, hid, rows, nlayer,
            alpha, ncores, a_dtype=a_dtype,
        )
    nc.compile()
    return nc


def make_in_maps(nodes_encs, W, b, adj, ncores=NCORES):
    n = adj.shape[0]
    rows = n // ncores
    emb = np.asarray(W).shape[0]
    hid = np.asarray(W).shape[1]
    # bias folded into an extra 128-row contraction block:
    # X' = [X | e0-block], W' = [W ; b-row-block]
    Wp = np.zeros((emb + 128, hid), dtype=np.float32)
    Wp[:emb] = np.asarray(W, dtype=np.float32)
    Wp[emb] = np.asarray(b, dtype=np.float32).reshape(-1)
    in_maps = []
    for c in range(ncores):
        sl = slice(c * rows, (c + 1) * rows)
        xtp = np.zeros((emb + 128, rows), dtype=np.float32)
        xtp[:emb] = np.asarray(nodes_encs)[sl, :].T
        xtp[emb] = 1.0
        in_maps.append(
            {
                "at": np.ascontiguousarray(np.asarray(adj)[sl, :].T, dtype=np.float32),
                "xt": xtp,
                "w": Wp,
            }
        )
    return in_maps


def kernel(nodes_encs, W, b, adj, trace=True):
    global LAST_EXEC_NS, LAST_TRACE
    _install_ntff_hook()
    a_dtype = BF16 if os.environ.get("APPNP_A_BF16") else FP8
    nc = build(a_dtype=a_dtype)
    in_maps = make_in_maps(nodes_encs, W, b, adj)
    res = None
    if trace:
        try:
            # warmup execution absorbs NEFF-load / core-start skew
            run_bass_kernel_spmd(
                nc, in_maps, core_ids=list(range(NCORES)), trace=False
            )
            res = run_bass_kernel_spmd(
                nc, in_maps, core_ids=list(range(NCORES)), trace=True
            )
        except Exception:
            res = None
    if res is None:
        res = run_bass_kernel_spmd(
            nc, in_maps, core_ids=list(range(NCORES)), trace=False
        )
    LAST_EXEC_NS = res.exec_time_ns
    LAST_TRACE = getattr(res, "instructions_and_trace", None)
    out = np.concatenate([res.results[c]["out"] for c in range(NCORES)], axis=0)
    return out.astype(np.float32)
